# revision 1
# baseline (speedup 1.0000x reference)
"""Trainium2 Bass kernel for the AbstractQCP residual operator F @ W.

Math (reference):
    v = y - s; mask = (v >= 0)
    dx = wx; dy = mask*wy; dt = wt        (W = [wx; wy; wt], (n+m+1, K))
    o1 = P@dx + A.T@dy + q dt             (n, K)
    o2 = b dt - A@dx                      (m, K)
    o3 = (x.T P x) dt - (q + 2 P x)@dx - b@dy
    F  = [o1; o2 + (1-mask)*wy; o3]       (since dx==wx, dt==wt the -dPi+W
                                           residual cancels on the n/t blocks)

Sharding across 8 NeuronCores (pure SPMD, no device collectives):
  core i owns output rows: o1[512i:512(i+1)], o2[1024i:1024(i+1)], and a
  partial of o3 (host sums the 8 (1,256) partials).
  GEMM1: lhsT_B = [P[:,cols_i]; A[:,cols_i]; q_i] (12289+pad, 512) -- P
  symmetric so P[:,cols] == P[rows,:].T.  rhs = [W | e] with e=[x;0;0]
  (257 cols) so column 256 of the GEMM1 result is P_i @ x for free.
  GEMM2: lhsT_C = [-A[rows_i,:].T; b_i] (4097+pad, 1024), rhs = n-block
  rows of W plus the wt row.
  All matmul operands bf16 (host-cast), accumulation fp32 in PSUM.

All streamed operands are staged in DRAM K-tile-transposed -- shape
(128, ktiles*free) with element (p, k*free+c) = orig(k*128+p, c) -- so a
single DMA moves several K-tiles with >=4KB contiguous per partition.
"""

import numpy as np
import ml_dtypes
from contextlib import ExitStack

BF = ml_dtypes.bfloat16

N, M, KP = 4096, 8192, 256
NC = 8
NS, MS = N // NC, M // NC          # 512, 1024
F = KP + 1                         # 257: probes + aug column
KT1, KT2 = 97, 33                  # contraction tiles (128 rows each)
R1, R2 = KT1 * 128, KT2 * 128      # 12416, 4224 (zero-padded)

G1 = 8     # wa / bt K-tiles per DMA group
G2 = 4     # ct K-tiles per DMA group

_NC_CACHE = None


def _kt(a, ktiles, free):
    """(ktiles*128, free) row-major -> (128, ktiles*free) K-tile-transposed."""
    return np.ascontiguousarray(
        a.reshape(ktiles, 128, free).transpose(1, 0, 2).reshape(128, ktiles * free))


def _build_nc():
    from concourse import bacc, tile, mybir
    from concourse.alu_op_type import AluOpType as op

    dtb = mybir.dt.bfloat16
    dtf = mybir.dt.float32

    nc = bacc.Bacc("TRN2", target_bir_lowering=False, debug=False)

    def din(name, shape, dt):
        return nc.dram_tensor(name, list(shape), dt, kind="ExternalInput").ap()

    bt = din("bt", (128, KT1 * NS), dtb)    # GEMM1 lhsT, K-tile-transposed
    ct = din("ct", (128, KT2 * MS), dtb)    # GEMM2 lhsT, K-tile-transposed
    wa = din("wa", (128, KT1 * F), dtb)     # [W | e] rhs, K-tile-transposed
    yt = din("yt", (128, 64), dtf)          # y in (partition, m-tile) layout
    st = din("st", (128, 64), dtf)
    yto = din("yto", (128, 8), dtf)         # own m-shard slices of yt/st
    sto = din("sto", (128, 8), dtf)
    nq = din("nq", (128, 4), dtf)           # -q_i
    xv = din("xv", (128, 4), dtb)           # x_i
    nb = din("nb", (128, 8), dtb)           # -b_i
    wown = din("wown", (128, 8 * F), dtb)   # own wy rows, K-tile-transposed
    xw = din("xw", (128, 4 * F), dtb)       # own wx rows, K-tile-transposed
    out1 = nc.dram_tensor("out1", [128, 4 * KP], dtb, kind="ExternalOutput").ap()
    out2 = nc.dram_tensor("out2", [128, 8 * KP], dtb, kind="ExternalOutput").ap()
    out3 = nc.dram_tensor("out3", [1, KP], dtf, kind="ExternalOutput").ap()

    with tile.TileContext(nc) as tc, ExitStack() as ctx:
        dpool = ctx.enter_context(tc.tile_pool(name="d", bufs=1))
        wpool = ctx.enter_context(tc.tile_pool(name="w", bufs=4))
        cpool = ctx.enter_context(tc.tile_pool(name="c", bufs=4))
        spool = ctx.enter_context(tc.tile_pool(name="s", bufs=1))
        opool = ctx.enter_context(tc.tile_pool(name="o", bufs=1))
        pspool = ctx.enter_context(tc.tile_pool(name="ps", bufs=8, space="PSUM"))

        # --- small vectors + masks: emitted as a deferred block (at the
        # end of loop step k==8) so their DMA triggers don't delay the
        # first weight chunks. None of them is consumed before ~k=24.
        sm = {}

        def emit_smalls():
            ytb = spool.tile((128, 64), dtf, tag="ytb")
            nc.scalar.dma_start(ytb, yt)
            stb = spool.tile((128, 64), dtf, tag="stb")
            nc.scalar.dma_start(stb, st)
            v = spool.tile((128, 64), dtf, tag="v")
            nc.vector.tensor_sub(v, ytb, stb)
            mask = spool.tile((128, 64), dtf, tag="mask")
            nc.vector.tensor_scalar(mask, v, 0.0, None, op.is_ge)
            sm["mask"] = mask

            ytob = spool.tile((128, 8), dtf, tag="ytob")
            nc.scalar.dma_start(ytob, yto)
            stob = spool.tile((128, 8), dtf, tag="stob")
            nc.scalar.dma_start(stob, sto)
            vo = spool.tile((128, 8), dtf, tag="vo")
            nc.vector.tensor_sub(vo, ytob, stob)
            masko = spool.tile((128, 8), dtf, tag="masko")
            nc.vector.tensor_scalar(masko, vo, 0.0, None, op.is_ge)
            umo = spool.tile((128, 8), dtf, tag="umo")
            nc.vector.tensor_scalar(umo, masko, -1.0, 1.0, op.mult, op.add)
            sm["umo"] = umo

            nqb = spool.tile((128, 4), dtf, tag="nqb")
            nc.scalar.dma_start(nqb, nq)
            sm["nqb"] = nqb
            xvb = spool.tile((128, 4), dtb, tag="xvb")
            nc.scalar.dma_start(xvb, xv)
            sm["xvb"] = xvb
            nbb = spool.tile((128, 8), dtb, tag="nbb")
            nc.scalar.dma_start(nbb, nb)
            sm["nbb"] = nbb

            wosb = spool.tile((128, 8 * F), dtb, tag="wosb")
            nc.scalar.dma_start(wosb, wown)
            sm["wosb"] = wosb
            wm = []
            for t_i in range(8):
                mt = spool.tile((128, F), dtb, tag=f"wm{t_i}")
                nc.vector.tensor_scalar_mul(mt, wosb[:, t_i * F:(t_i + 1) * F],
                                            masko[:, t_i:t_i + 1])
                wm.append(mt)
            sm["wm"] = wm
            xwsb = spool.tile((128, 4 * F), dtb, tag="xwsb")
            nc.scalar.dma_start(xwsb, xw)
            sm["xwsb"] = xwsb

        # --- resident rhs tiles; group DMAs are emitted just-in-time
        # inside the unified loop. First groups are small so the PE's
        # first inputs land quickly after the preamble.
        WB = [0, 1, 2, 4, 8, 16, 24, 32, 40, 48, 56, 64, 72, 80, 88, 96, 97]
        BB = [0, 1, 2, 4, 8, 16, 24, 32, 40, 48, 56, 64, 72, 80, 88, 96, 97]  # bt chunk bounds
        CB = [0, 1, 4, 8, 12, 16, 20, 24, 28, 32, 33]
        k2g = {}
        for g in range(len(WB) - 1):
            for k in range(WB[g], WB[g + 1]):
                k2g[k] = g
        k2b = {}
        for g in range(len(BB) - 1):
            for k in range(BB[g], BB[g + 1]):
                k2b[k] = g
        j2c = {}
        for c in range(len(CB) - 1):
            for j in range(CB[c], CB[c + 1]):
                j2c[j] = c

        wag = [None] * (len(WB) - 1)
        dm = [None] * 64

        def load_wag(g):
            k0, k1 = WB[g], WB[g + 1]
            t = dpool.tile((128, (k1 - k0) * F), dtb, tag=f"wag{g}", name=f"wag{g}")
            nc.sync.dma_start(t, wa[:, k0 * F:k1 * F])
            wag[g] = t
            for j in range(k0, k1):
                jm = j - 32
                if 0 <= jm < 64:
                    mt = dpool.tile((128, F), dtb, tag=f"dm{jm}", name=f"dm{jm}")
                    nc.vector.tensor_scalar_mul(
                        mt, t[:, (j - k0) * F:(j - k0 + 1) * F], sm["mask"][:, jm:jm + 1])
                    dm[jm] = mt

        def dslice(k):
            g = k2g[k]
            return wag[g][:, (k - WB[g]) * F:(k - WB[g] + 1) * F]

        def rhs1(k):
            return dslice(k) if (k < 32 or k == 96) else dm[k - 32]

        # --- unified interleaved loop ---------------------------------
        # GEMM1 k-tile per step; GEMM2 tick j at step 5+round(2.3j) --
        # sparse early (while wa streams) and stopping at step 79 so the
        # f2 evictions + output DMA overlap the last GEMM1 steps.
        # psum: gemm1 4 banks (128,257); gemm2 4 banks (128,512) holding
        # two 256-wide accumulators each (bank-shared start/stop flags).
        ps1 = [pspool.tile((128, F), dtf, tag="ps", name=f"ps1_{m}") for m in range(4)]
        ps2 = [pspool.tile((128, 512), dtf, tag="ps", name=f"ps2_{u}") for u in range(4)]

        tick_at = {(0 if j == 0 else 12 + round(1.94 * j)): j for j in range(KT2)}

        load_wag(0)
        btt = None
        btt_k0 = 0
        ctt = None
        ctt_j0 = 0
        px = spool.tile((128, 4), dtb, tag="px")
        cf = spool.tile((128, 4), dtb, tag="cf")
        pso3 = None
        for k in range(KT1):
            if k == 0 or k2g[k] != k2g[k - 1]:
                g = k2g[k]
                if g + 1 < len(WB) - 2:
                    load_wag(g + 1)
            if k == 0 or k2b[k] != k2b[k - 1]:
                g = k2b[k]
                k0, k1 = BB[g], BB[g + 1]
                btt = wpool.tile((128, G1 * NS), dtb, tag="bt",
                                 name=f"btt{g}", padded_shape=(128, G1 * NS))
                btt_k0 = k0
                nc.sync.dma_start(btt[:, :(k1 - k0) * NS], bt[:, k0 * NS:k1 * NS])
            if k == 40:
                load_wag(len(WB) - 2)   # d[96] (wt row): needed at tick 32
            j = tick_at.get(k)
            if j is not None:
                if j == 0 or j2c[j] != j2c[j - 1]:
                    c = j2c[j]
                    j0, j1 = CB[c], CB[c + 1]
                    ctt = cpool.tile((128, G2 * MS), dtb, tag="ct",
                                     name=f"ctt{c}", padded_shape=(128, G2 * MS))
                    ctt_j0 = j0
                    nc.gpsimd.dma_start(ctt[:, :(j1 - j0) * MS], ct[:, j0 * MS:j1 * MS])
                rhs2 = dslice(j)[:, 0:KP] if j < 32 else dslice(96)[:, 0:KP]
                jo = j - ctt_j0
                for t_i in range(8):
                    # Bank sharing: slice t%2==0 owns start (clears whole
                    # bank); slice t%2==1's first write lands on cleared
                    # has_written bits so flags=0 overwrites correctly.
                    # Only the bank's last write carries stop.
                    nc.tensor.matmul(
                        ps2[t_i // 2][:, (t_i % 2) * KP:(t_i % 2 + 1) * KP],
                        ctt[:, jo * MS + t_i * 128:jo * MS + (t_i + 1) * 128],
                        rhs2, start=(j == 0 and t_i % 2 == 0),
                        stop=(j == KT2 - 1 and t_i % 2 == 1))
                if j == KT2 - 1:
                    # gemm2 done: evict f2 while gemm1 finishes; stage all
                    # 8 tiles contiguously so one DMA moves 8KB/partition
                    ob2 = opool.tile((128, 8 * KP), dtb, tag="ob2")
                    for t_i in range(8):
                        # f2 = (wy * (1-mask)) + o2pre
                        nc.vector.scalar_tensor_tensor(
                            ob2[:, t_i * KP:(t_i + 1) * KP],
                            sm["wosb"][:, t_i * F:t_i * F + KP], sm["umo"][:, t_i:t_i + 1],
                            ps2[t_i // 2][:, (t_i % 2) * KP:(t_i % 2 + 1) * KP],
                            op.mult, op.add)
                    nc.scalar.dma_start(out2, ob2)
            if k == 88:
                # o3's -b@dy half: no gemm1 dependency, runs while psum
                # slots freed by the f2 evictions are available
                pso3 = pspool.tile((1, F), dtf, tag="ps", name="pso3")
                for t_i in range(8):
                    nc.tensor.matmul(pso3, sm["nbb"][:, t_i:t_i + 1], sm["wm"][t_i],
                                     start=(t_i == 0), stop=False)
            if k == 81:
                # phase2 accumulators take the banks gemm2 just freed
                ps1b = [pspool.tile((128, KP), dtf, tag="ps", name=f"ps1b_{m}")
                        for m in range(4)]
            jb = k - btt_k0
            for m in range(4):
                if k <= 80:
                    nc.tensor.matmul(
                        ps1[m],
                        btt[:, jb * NS + m * 128:jb * NS + (m + 1) * 128],
                        rhs1(k), start=(k == 0), stop=(k == 80))
                else:
                    nc.tensor.matmul(
                        ps1b[m],
                        btt[:, jb * NS + m * 128:jb * NS + (m + 1) * 128],
                        rhs1(k)[:, 0:KP], start=(k == 81), stop=(k == KT1 - 1))
            if k == 16:
                emit_smalls()
            if k == 80:
                # phase1 eviction mid-loop: Px column is complete (aug col
                # is zero past the n block), so the whole o3 chain can run
                # inside the loop
                pr = []
                for m in range(4):
                    nc.vector.tensor_copy(px[:, m:m + 1], ps1[m][:, KP:KP + 1])
                    # cf = -(q + 2 Px) = (Px * -2) + (-q)
                    nc.vector.scalar_tensor_tensor(
                        cf[:, m:m + 1], ps1[m][:, KP:KP + 1], -2.0,
                        sm["nqb"][:, m:m + 1], op.mult, op.add)
                for m in range(4):
                    t = spool.tile((128, KP), dtf, tag=f"pr{m}")
                    nc.vector.tensor_copy(t, ps1[m][:, 0:KP])
                    pr.append(t)
            if k == 86:
                psxx = pspool.tile((1, 1), dtf, tag="ps")
                for j3 in range(4):
                    nc.tensor.matmul(psxx, px[:, j3:j3 + 1], sm["xvb"][:, j3:j3 + 1],
                                     start=(j3 == 0), stop=(j3 == 3))
            if k == 90:
                for j3 in range(4):
                    nc.tensor.matmul(pso3, cf[:, j3:j3 + 1],
                                     sm["xwsb"][:, j3 * F:(j3 + 1) * F],
                                     start=False, stop=(j3 == 3))
            if k == 92:
                o3f = opool.tile((1, KP), dtf, tag="o3f")
                # o3 = wt * xPx + (cf@dx + (-b)@dy)
                nc.vector.scalar_tensor_tensor(o3f, dslice(96)[0:1, 0:KP],
                                               psxx[0:1, 0:1], pso3[0:1, 0:KP],
                                               op.mult, op.add)
                nc.scalar.dma_start(out3, o3f)

        # --- final combine: o1 = phase1 partial + phase2 psum ---------
        ob1 = opool.tile((128, 4 * KP), dtb, tag="ob1")
        for m in range(4):
            nc.vector.tensor_tensor(ob1[:, m * KP:(m + 1) * KP], pr[m],
                                    ps1b[m][:, 0:KP], op.add)
        nc.scalar.dma_start(out1, ob1)

    nc.compile()
    return nc


def _get_nc():
    global _NC_CACHE
    if _NC_CACHE is None:
        _NC_CACHE = _build_nc()
    return _NC_CACHE


def _prep_in_maps(P, A, q, b, x, y, s, W):
    P = np.asarray(P, np.float32)
    A = np.asarray(A, np.float32)
    q = np.asarray(q, np.float32)
    b = np.asarray(b, np.float32)
    x = np.asarray(x, np.float32)
    y = np.asarray(y, np.float32)
    s = np.asarray(s, np.float32)
    W = np.asarray(W, np.float32)

    Pb, Ab = P.astype(BF), A.astype(BF)
    qb, bb, xb, Wb = q.astype(BF), b.astype(BF), x.astype(BF), W.astype(BF)

    wa0 = np.zeros((R1, F), BF)
    wa0[:N + M + 1, :KP] = Wb
    wa0[:N, KP] = xb
    wa = _kt(wa0, KT1, F)
    yt = np.ascontiguousarray(y.reshape(64, 128).T)
    st_ = np.ascontiguousarray(s.reshape(64, 128).T)

    in_maps = []
    for i in range(NC):
        ncol = slice(i * NS, (i + 1) * NS)
        mrow = slice(i * MS, (i + 1) * MS)
        bt0 = np.zeros((R1, NS), BF)
        bt0[:N] = Pb[:, ncol]
        bt0[N:N + M] = Ab[:, ncol]
        bt0[N + M] = qb[ncol]
        ct0 = np.zeros((R2, MS), BF)
        ct0[:N] = (-A[mrow].T).astype(BF)
        ct0[N] = bb[mrow]
        in_maps.append(dict(
            bt=_kt(bt0, KT1, NS), ct=_kt(ct0, KT2, MS), wa=wa,
            yt=yt, st=st_,
            yto=np.ascontiguousarray(yt[:, 8 * i:8 * i + 8]),
            sto=np.ascontiguousarray(st_[:, 8 * i:8 * i + 8]),
            nq=np.ascontiguousarray((-q[ncol]).reshape(4, 128).T),
            xv=np.ascontiguousarray(x[ncol].reshape(4, 128).T.astype(BF)),
            nb=np.ascontiguousarray((-b[mrow]).reshape(8, 128).T.astype(BF)),
            wown=_kt(wa0[N + i * MS:N + (i + 1) * MS], 8, F),
            xw=_kt(wa0[i * NS:(i + 1) * NS], 4, F),
        ))
    return in_maps


def _assemble(results):
    Fo = np.empty((N + M + 1, KP), np.float32)
    o3 = np.zeros((KP,), np.float32)
    for i in range(NC):
        o1 = np.asarray(results[i]["out1"], np.float32)     # (128, 4*KP)
        o2 = np.asarray(results[i]["out2"], np.float32)     # (128, 8*KP)
        Fo[i * NS:(i + 1) * NS] = (
            o1.reshape(128, 4, KP).transpose(1, 0, 2).reshape(NS, KP))
        Fo[N + i * MS:N + (i + 1) * MS] = (
            o2.reshape(128, 8, KP).transpose(1, 0, 2).reshape(MS, KP))
        o3 += np.asarray(results[i]["out3"], np.float32)[0]
    Fo[N + M] = o3
    return Fo


def _run_sharded(inputs, trace=False, trace_kwargs=None):
    from concourse import bass_utils
    nc = _get_nc()
    in_maps = _prep_in_maps(**inputs)
    res = bass_utils.run_bass_kernel_spmd(
        nc, in_maps, core_ids=list(range(NC)), trace=trace,
        **(trace_kwargs or {}))
    return _assemble(res.results), res


def kernel(**inputs) -> np.ndarray:
    out, _ = _run_sharded(inputs, trace=False)
    return out



# revision 7
# speedup vs baseline: 1.0549x; 1.0549x over previous
"""Trainium2 Bass kernel for the AbstractQCP residual operator F @ W.

Math (reference):
    v = y - s; mask = (v >= 0)
    dx = wx; dy = mask*wy; dt = wt        (W = [wx; wy; wt], (n+m+1, K))
    o1 = P@wx + A.T@dy + q wt             (n, K)
    o2 = b wt - A@wx                      (m, K)
    o3 = (x.T P x) wt - (q + 2 P x)@wx - b@dy
    F  = [o1; o2 + (1-mask)*wy; o3]

Design (per core i of 8, pure SPMD, host gathers):
  core i owns o1 rows [512i,512(i+1)) and o2 rows [1024i,1024(i+1)).
  - Host precomputes: mask, the row-compacted A.T@dy operands (only the
    ~m/2 rows where mask=1 contribute), Px = P@x, xTPx, cf = -(q+2Px).
  - G1P (bf16): lhsT=[q_i; P[:,cols_i]] K-tiled (33 tiles, tile0 = q row),
    rhs=[wt; wx] K-tiled.  P symmetric so P[:,cols].T = P[cols,:].
  - G1A (fp8 e3m4): lhsT = SA*A[maskrows, cols_i] K-tiled (compacted),
    rhs = SW*wy[maskrows] K-tiled.  o1 = psP + psA/(SA*SW).
  - G2 (fp8): lhsT = -SA*A[rows_i,:].T K-tiled (32 tiles),
    rhs = SW*wx (cast on device from the bf16 W stream). b wt added via a
    tiny bf16 contraction-1 matmul with lhsT = b*SA*SW.
    o2 = ps2/(SA*SW) + (1-mask)*wy.  Optional e4m3+DoubleRow mode.
  - o3 partial per core: cf@wx_i + (-b_i)@(mask*wy_i); host adds xTPx*wt.
  PSUM: 2 banks G1P + 2 banks G1A + 4 banks G2 (2 accumulators per bank,
  bank-shared start/stop flags).

All streamed operands staged in DRAM K-tile-transposed: (128, ktiles*free)
with element (p, k*free+c) = orig(k*128+p, c).
"""

import numpy as np
import ml_dtypes
from contextlib import ExitStack

BF = ml_dtypes.bfloat16
E3 = ml_dtypes.float8_e3m4
E4 = ml_dtypes.float8_e4m3

N, M, KP = 4096, 8192, 256
NC = 8
NS, MS = N // NC, M // NC          # 512, 1024
KTP = 33                           # G1P k-tiles: [q/wt tile] + 32 P tiles
KT2 = 32                           # G2 k-tiles (full n contraction)

G2_DR = False                      # G2 as e4m3 + DoubleRow (else e3m4 normal)

_NC_CACHE = {}


def _kt(a, ktiles, free):
    """(ktiles*128, free) row-major -> (128, ktiles*free) K-tile-transposed."""
    return np.ascontiguousarray(
        a.reshape(ktiles, 128, free).transpose(1, 0, 2).reshape(128, ktiles * free))


def _build_nc(kta, g2_dr, c1, c2):
    from concourse import bacc, tile, mybir
    from concourse.alu_op_type import AluOpType as op

    dtb = mybir.dt.bfloat16
    dtf = mybir.dt.float32
    dt8a = mybir.dt.float8e3
    dt8c = mybir.dt.float8e4 if g2_dr else mybir.dt.float8e3
    pm = mybir.MatmulPerfMode.DoubleRow if g2_dr else None

    nc = bacc.Bacc("TRN2", target_bir_lowering=False, debug=False)

    def din(name, shape, dt):
        return nc.dram_tensor(name, list(shape), dt, kind="ExternalInput").ap()

    pt = din("pt", (128, KTP * NS), dtb)      # [q; P cols] K-tiled
    wtb = din("wtb", (128, KTP * KP), dtb)    # [wt; wx] K-tiled
    at8 = din("at8", (128, kta * NS), dt8a)   # compacted SA*A rows, K-tiled
    dy8 = din("dy8", (128, kta * KP), dt8a)   # compacted SW*wy, K-tiled
    if g2_dr:
        ct8 = din("ct8", (128, KT2, MS), dt8c)
    else:
        ct8 = din("ct8", (128, KT2 * MS), dt8c)
    bxd = din("bx", (1, MS), dtb)             # b_i * SA2 * SW
    cfd = din("cfb", (128, 4), dtb)           # cf shard, (4,128).T layout
    nbd = din("nbb", (128, 8), dtb)           # -b_i, (8,128).T layout
    ytd = din("yto", (128, 8), dtf)
    std = din("sto", (128, 8), dtf)
    wod = din("wosb", (128, 8 * KP), dtb)     # own wy rows K-tiled
    xwd = din("xw", (128, 4 * KP), dtb)       # own wx rows K-tiled
    out1 = nc.dram_tensor("out1", [128, 4 * KP], dtb, kind="ExternalOutput").ap()
    out2 = nc.dram_tensor("out2", [128, 8 * KP], dtb, kind="ExternalOutput").ap()
    out3 = nc.dram_tensor("out3", [1, KP], dtf, kind="ExternalOutput").ap()

    NSTEP = KTP + kta
    # group boundaries (k-tiles)
    WG = [0, 1, 3, 7, 15, 24, 33]
    AG = [0, 8, 16, 24, kta] if kta > 24 else [0, 8, 16, kta]
    AG = sorted(set(min(b, kta) for b in AG))
    YG = sorted(set(min(b, kta) for b in [0, 16, kta]))
    CG = [0, 4, 12, 20, 28, 32]

    def g_of(bounds):
        m = {}
        for g in range(len(bounds) - 1):
            for k in range(bounds[g], bounds[g + 1]):
                m[k] = g
        return m

    wg_of, ag_of, yg_of, cg_of = g_of(WG), g_of(AG), g_of(YG), g_of(CG)

    # G2 tick schedule: e3 -> 32 single-ktile ticks; DR -> 16 pair ticks
    nticks = KT2 // 2 if g2_dr else KT2
    last_tick_step = NSTEP - 10
    tick_step = [2 + round(t * (last_tick_step - 2) / (nticks - 1))
                 for t in range(nticks)]
    t2s = {}
    for t, s_ in enumerate(tick_step):
        t2s.setdefault(s_, []).append(t)
    # ct group g prefetch step: 5 steps before its first tick
    ct_load_step = {}
    for g in range(len(CG) - 1):
        first_tick = CG[g] // (2 if g2_dr else 1)
        ct_load_step.setdefault(max(0, tick_step[min(first_tick, nticks - 1)] - 5),
                                []).append(g)

    with tile.TileContext(nc) as tc, ExitStack() as ctx:
        dpool = ctx.enter_context(tc.tile_pool(name="d", bufs=1))
        wpool = ctx.enter_context(tc.tile_pool(name="w", bufs=3))
        ppool = ctx.enter_context(tc.tile_pool(name="p", bufs=3))
        apool = ctx.enter_context(tc.tile_pool(name="a", bufs=2))
        ypool = ctx.enter_context(tc.tile_pool(name="y", bufs=2))
        cpool = ctx.enter_context(tc.tile_pool(name="c", bufs=2))
        opool = ctx.enter_context(tc.tile_pool(name="o", bufs=1))
        pspool = ctx.enter_context(tc.tile_pool(name="ps", bufs=8, space="PSUM"))

        # PSUM: 8 banks, each (128, 512) fp32 holding two 256-wide slots
        psP = [pspool.tile((128, 2 * KP), dtf, tag="ps", name=f"psP{i}") for i in range(2)]
        psA = [pspool.tile((128, 2 * KP), dtf, tag="ps", name=f"psA{i}") for i in range(2)]
        ps2 = [pspool.tile((128, 2 * KP), dtf, tag="ps", name=f"ps2{i}") for i in range(4)]

        def pslot(tiles, t):
            return tiles[t // 2][:, (t % 2) * KP:(t % 2 + 1) * KP]

        # streaming group tiles
        wtbg, ptg, atg, dyg, ctg = {}, {}, {}, {}, {}

        def load_w(g):
            k0, k1 = WG[g], WG[g + 1]
            t = wpool.tile((128, (k1 - k0) * KP), dtb, tag="wtb", name=f"wtbg{g}",
                           padded_shape=(128, 9 * KP))
            nc.scalar.dma_start(t, wtb[:, k0 * KP:k1 * KP])
            wtbg[g] = t

        def load_p(g):
            k0, k1 = WG[g], WG[g + 1]
            t = ppool.tile((128, (k1 - k0) * NS), dtb, tag="pt", name=f"ptg{g}",
                           padded_shape=(128, 9 * NS))
            nc.sync.dma_start(t, pt[:, k0 * NS:k1 * NS])
            ptg[g] = t

        def load_a(g):
            k0, k1 = AG[g], AG[g + 1]
            t = apool.tile((128, (k1 - k0) * NS), dt8a, tag="at",
                           name=f"atg{g}", padded_shape=(128, 9 * NS))
            nc.sync.dma_start(t, at8[:, k0 * NS:k1 * NS])
            atg[g] = t

        def load_y(g):
            k0, k1 = YG[g], YG[g + 1]
            t = ypool.tile((128, (k1 - k0) * KP), dt8a, tag="dy", name=f"dyg{g}",
                           padded_shape=(128, 17 * KP))
            nc.gpsimd.dma_start(t, dy8[:, k0 * KP:k1 * KP])
            dyg[g] = t

        def load_c(g):
            j0, j1 = CG[g], CG[g + 1]
            if g2_dr:
                t = cpool.tile((128, 8, MS), dt8c, tag="ct", name=f"ctg{g}")
                nc.gpsimd.dma_start(t[:, 0:j1 - j0, :], ct8[:, j0:j1, :])
            else:
                t = cpool.tile((128, 8 * MS), dt8c, tag="ct", name=f"ctg{g}",
                               padded_shape=(128, 8 * MS))
                nc.gpsimd.dma_start(t[:, :(j1 - j0) * MS], ct8[:, j0 * MS:j1 * MS])
            ctg[g] = t

        # wx8: 16 resident fp8 pair tiles, cast on device from the wtb stream
        wx8p = [dpool.tile((128, 2, KP), dt8c, tag=f"wx8_{p_}", name=f"wx8_{p_}")
                for p_ in range(KT2 // 2)]

        sm = {}

        def emit_smalls():
            for nm, dref, shp, dt in (("bxs", bxd, (1, MS), dtb),
                                      ("cfs", cfd, (128, 4), dtb),
                                      ("nbs", nbd, (128, 8), dtb),
                                      ("yts", ytd, (128, 8), dtf),
                                      ("sts", std, (128, 8), dtf)):
                t = dpool.tile(shp, dt, tag=nm, name=nm)
                nc.scalar.dma_start(t, dref)
                sm[nm] = t
            t = dpool.tile((128, 8 * KP), dtb, tag="wos", name="wos")
            nc.scalar.dma_start(t, wod)
            sm["wos"] = t
            t = dpool.tile((128, 4 * KP), dtb, tag="xws", name="xws")
            nc.scalar.dma_start(t, xwd)
            sm["xws"] = t

        def emit_masks():
            vo = dpool.tile((128, 8), dtf, tag="vo", name="vo")
            nc.gpsimd.tensor_sub(vo, sm["yts"], sm["sts"])
            masko = dpool.tile((128, 8), dtf, tag="masko", name="masko")
            nc.gpsimd.tensor_scalar(masko, vo, 0.0, None, op.is_ge)
            umo = dpool.tile((128, 8), dtf, tag="umo", name="umo")
            nc.gpsimd.tensor_scalar(umo, masko, -1.0, 1.0, op.mult, op.add)
            sm["umo"] = umo

        def emit_wom():
            wom = dpool.tile((128, 8 * KP), dtb, tag="wom", name="wom")
            wmt = dpool.tile((128, 8 * KP), dtb, tag="wmt", name="wmt")
            for t_i in range(8):
                sl = slice(t_i * KP, (t_i + 1) * KP)
                nc.gpsimd.tensor_scalar_mul(wom[:, sl], sm["wos"][:, sl],
                                            sm["umo"][:, t_i:t_i + 1])
            for t_i in range(8):
                sl = slice(t_i * KP, (t_i + 1) * KP)
                nc.gpsimd.tensor_sub(wmt[:, sl], sm["wos"][:, sl], wom[:, sl])
            sm["wom"] = wom
            sm["wmt"] = wmt

        prp = dpool.tile((128, 4 * KP), dtf, tag="prp", name="prp")
        ob1 = opool.tile((128, 4 * KP), dtb, tag="ob1", name="ob1")
        ob2 = opool.tile((128, 8 * KP), dtb, tag="ob2", name="ob2")

        load_w(0)
        load_p(0)
        load_w(1)
        load_p(1)

        done_ticks = 0
        at_started = False
        for k in range(NSTEP):
            is_p = k < KTP
            kk = k if is_p else k - KTP

            # --- JIT stream prefetch ---
            if is_p:
                g = wg_of[kk]
                if kk == WG[g] and g + 2 <= len(WG) - 2:
                    load_w(g + 2)
                    load_p(g + 2)
            else:
                g = ag_of[kk]
                if kk == AG[g] and g + 1 <= len(AG) - 2:
                    load_a(g + 1)
                if kk == YG[yg_of[kk]] and yg_of[kk] + 1 <= len(YG) - 2:
                    load_y(yg_of[kk] + 1)
            if k == KTP - 8:
                load_a(0)
                load_y(0)
                at_started = True
            for g in ct_load_step.get(k, []):
                load_c(g)

            # --- G1 matmuls (4 m-blocks into 2 shared banks) ---
            if is_p:
                g = wg_of[kk]
                rhs = wtbg[g][:, (kk - WG[g]) * KP:(kk - WG[g] + 1) * KP]
                pg = wg_of[kk]
                lt = ptg[pg]
                jo = kk - WG[pg]
                for m in range(4):
                    nc.tensor.matmul(
                        pslot(psP, m),
                        lt[:, jo * NS + m * 128:jo * NS + (m + 1) * 128],
                        rhs, start=(kk == 0 and m % 2 == 0),
                        stop=(kk == KTP - 1 and m % 2 == 1))
            else:
                g = ag_of[kk]
                yg = yg_of[kk]
                rhs = dyg[yg][:, (kk - YG[yg]) * KP:(kk - YG[yg] + 1) * KP]
                lt = atg[g]
                jo = kk - AG[g]
                for m in range(4):
                    nc.tensor.matmul(
                        pslot(psA, m),
                        lt[:, jo * NS + m * 128:jo * NS + (m + 1) * 128],
                        rhs, start=(kk == 0 and m % 2 == 0),
                        stop=(kk == kta - 1 and m % 2 == 1))

            # --- b (x) wt into ps2 (owns the start flags) ---
            if k == 1:
                for t_i in range(8):
                    nc.tensor.matmul(
                        pslot(ps2, t_i),
                        sm["bxs"][0:1, t_i * 128:(t_i + 1) * 128],
                        wtbg[0][0:1, 0:KP],
                        start=(t_i % 2 == 0), stop=False)

            # --- device cast of wx -> fp8 (vector) ---
            if is_p and 1 <= kk <= KT2:
                j = kk - 1
                g = wg_of[kk]
                src = wtbg[g][:, (kk - WG[g]) * KP:(kk - WG[g] + 1) * KP]
                nc.vector.tensor_scalar_mul(wx8p[j // 2][:, j % 2, :], src, SW_DEV)

            # --- G2 ticks ---
            for t in t2s.get(k, []):
                if g2_dr:
                    pair = t
                    g = cg_of[2 * pair]
                    jo = 2 * pair - CG[g]
                    for t_i in range(8):
                        nc.tensor.matmul(
                            pslot(ps2, t_i),
                            ctg[g][:, jo:jo + 2, t_i * 128:(t_i + 1) * 128],
                            wx8p[pair][:, 0:2, 0:KP],
                            start=False,
                            stop=(pair == KT2 // 2 - 1 and t_i % 2 == 1),
                            perf_mode=pm)
                else:
                    j = t
                    g = cg_of[j]
                    jo = j - CG[g]
                    for t_i in range(8):
                        nc.tensor.matmul(
                            pslot(ps2, t_i),
                            ctg[g][:, jo * MS + t_i * 128:jo * MS + (t_i + 1) * 128],
                            wx8p[j // 2][:, j % 2, :],
                            start=False,
                            stop=(j == KT2 - 1 and t_i % 2 == 1))
                done_ticks += 1

            # --- deferred small loads + mask computation ---
            if k == 0:
                emit_smalls()
            if k == 8:
                emit_masks()
            if k == 26:
                emit_wom()

            # --- evict psP to SBUF right after G1P finishes ---
            if k == KTP:
                for m in range(4):
                    nc.vector.tensor_copy(prp[:, m * KP:(m + 1) * KP], pslot(psP, m))

            # --- o2 eviction once G2 is done ---
            if done_ticks == nticks:
                done_ticks = -1  # fire once
                for t_i in range(8):
                    sl = slice(t_i * KP, (t_i + 1) * KP)
                    nc.vector.scalar_tensor_tensor(
                        ob2[:, sl], pslot(ps2, t_i), c2, sm["wom"][:, sl],
                        op.mult, op.add)
                nc.scalar.dma_start(out2, ob2)
                # o3 partial: (-b)@dy + cf@wx over own shards
                pso3 = pspool.tile((1, KP), dtf, tag="ps", name="pso3")
                for t_i in range(8):
                    nc.tensor.matmul(pso3, sm["nbs"][:, t_i:t_i + 1],
                                     sm["wmt"][:, t_i * KP:(t_i + 1) * KP],
                                     start=(t_i == 0), stop=False)
                for j3 in range(4):
                    nc.tensor.matmul(pso3, sm["cfs"][:, j3:j3 + 1],
                                     sm["xws"][:, j3 * KP:(j3 + 1) * KP],
                                     start=False, stop=(j3 == 3))
                o3f = opool.tile((1, KP), dtf, tag="o3f", name="o3f")
                nc.vector.tensor_copy(o3f, pso3)
                nc.scalar.dma_start(out3, o3f)

        # --- final o1 combine: psP (via prp) + psA/(SA*SW) ---
        for m in range(4):
            nc.vector.scalar_tensor_tensor(
                ob1[:, m * KP:(m + 1) * KP], pslot(psA, m), c1,
                prp[:, m * KP:(m + 1) * KP], op.mult, op.add)
        nc.scalar.dma_start(out1, ob1)

    nc.compile()
    return nc


SW_DEV = 1.0  # device-side wx scale (set by _prep before build)


def _pow2_scale(std, mx, limit):
    if not np.isfinite(std) or std <= 0:
        return 1.0
    s = 2.0 ** round(np.log2(1.0 / std))
    while mx * s > limit:
        s *= 0.5
    return s


def _get_nc(key):
    if key not in _NC_CACHE:
        _NC_CACHE[key] = _build_nc(*key)
    return _NC_CACHE[key]


def _prep(P, A, q, b, x, y, s, W):
    global SW_DEV
    P = np.asarray(P, np.float32)
    A = np.asarray(A, np.float32)
    q = np.asarray(q, np.float32)
    b = np.asarray(b, np.float32)
    x = np.asarray(x, np.float32)
    y = np.asarray(y, np.float32)
    s = np.asarray(s, np.float32)
    W = np.asarray(W, np.float32)

    mb = (y - s) >= 0
    idx = np.nonzero(mb)[0]
    mp = max(1, len(idx))
    kta = (mp + 127) // 128

    wx, wy, wt = W[:N], W[N:N + M], W[N + M:]
    e4 = G2_DR
    SA = _pow2_scale(A.std(), np.abs(A).max(), 200.0 if e4 else 14.0)
    SW = _pow2_scale(1.0, np.abs(W).max(), 200.0 if e4 else 14.0)
    SW_DEV = SW
    c1 = 1.0 / (SA * SW)
    c2 = 1.0 / (SA * SW)

    Px = P @ x
    xPx = float(x @ Px)
    cf = -(q + 2.0 * Px)

    # shared (core-independent) staging
    wtb_full = np.zeros((KTP * 128, KP), BF)
    wtb_full[0] = wt[0]
    wtb_full[128:128 + N] = wx.astype(BF)
    wtb_h = _kt(wtb_full, KTP, KP)

    Ac = A[idx]                       # (mp, N)
    at_q = (Ac * SA).astype(E3)       # quantize once, slice per core
    dy_full = np.zeros((kta * 128, KP), E3)
    dy_full[:mp] = (wy[idx] * SW).astype(E3)
    dy_h = _kt(dy_full, kta, KP)

    E4c = E4 if e4 else E3
    in_maps = []
    for i in range(NC):
        ncol = slice(i * NS, (i + 1) * NS)
        mrow = slice(i * MS, (i + 1) * MS)
        pt0 = np.zeros((KTP * 128, NS), BF)
        pt0[0] = q[ncol].astype(BF)
        pt0[128:128 + N] = P[:, ncol].astype(BF)
        at0 = np.zeros((kta * 128, NS), E3)
        at0[:mp] = at_q[:, ncol]
        ct0 = (-(SA * A[mrow].T)).astype(E4c)          # (N, MS)
        ct_h = _kt(ct0, KT2, MS)
        if G2_DR:
            ct_h = ct_h.reshape(128, KT2, MS)
        yto = np.ascontiguousarray(y[mrow].reshape(8, 128).T)
        sto = np.ascontiguousarray(s[mrow].reshape(8, 128).T)
        in_maps.append(dict(
            pt=_kt(pt0, KTP, NS), wtb=wtb_h,
            at8=_kt(at0, kta, NS), dy8=dy_h, ct8=ct_h,
            bx=np.ascontiguousarray((b[mrow] * SA * SW)[None, :].astype(BF)),
            cfb=np.ascontiguousarray(cf[ncol].reshape(4, 128).T.astype(BF)),
            nbb=np.ascontiguousarray((-b[mrow]).reshape(8, 128).T.astype(BF)),
            yto=yto, sto=sto,
            wosb=_kt(wy[mrow].astype(BF), 8, KP),
            xw=_kt(wx[ncol].astype(BF), 4, KP),
        ))
    return in_maps, kta, c1, c2, xPx, wt


def _assemble(results, xPx, wt):
    Fo = np.empty((N + M + 1, KP), np.float32)
    o3 = xPx * wt[0].astype(np.float32)
    for i in range(NC):
        o1 = np.asarray(results[i]["out1"], np.float32)     # (128, 4*KP)
        o2 = np.asarray(results[i]["out2"], np.float32)     # (128, 8*KP)
        Fo[i * NS:(i + 1) * NS] = (
            o1.reshape(128, 4, KP).transpose(1, 0, 2).reshape(NS, KP))
        Fo[N + i * MS:N + (i + 1) * MS] = (
            o2.reshape(128, 8, KP).transpose(1, 0, 2).reshape(MS, KP))
        o3 = o3 + np.asarray(results[i]["out3"], np.float32)[0]
    Fo[N + M] = o3
    return Fo


def _run_sharded(inputs, trace=False, trace_kwargs=None):
    from concourse import bass_utils
    in_maps, kta, c1, c2, xPx, wt = _prep(**inputs)
    nc = _get_nc((kta, G2_DR, c1, c2))
    res = bass_utils.run_bass_kernel_spmd(
        nc, in_maps, core_ids=list(range(NC)), trace=trace,
        **(trace_kwargs or {}))
    return _assemble(res.results, xPx, wt), res


def kernel(**inputs) -> np.ndarray:
    out, _ = _run_sharded(inputs, trace=False)
    return out


# revision 15
# speedup vs baseline: 1.0859x; 1.0293x over previous
"""Trainium2 Bass kernel for the AbstractQCP residual operator F @ W.

Math (reference):
    v = y - s; mask = (v >= 0)
    dx = wx; dy = mask*wy; dt = wt        (W = [wx; wy; wt], (n+m+1, K))
    o1 = P@wx + A.T@dy + q wt             (n, K)
    o2 = b wt - A@wx                      (m, K)
    o3 = (x.T P x) wt - (q + 2 P x)@wx - b@dy
    F  = [o1; o2 + (1-mask)*wy; o3]

Design (per core i of 8, pure SPMD, host gathers):
  core i owns o1 rows [512i,512(i+1)) and o2 rows [1024i,1024(i+1)).
  - Host precomputes: mask, the row-compacted A.T@dy operands (only the
    ~m/2 rows where mask=1 contribute), Px = P@x, xTPx, cf = -(q+2Px).
  - G1P (bf16): lhsT=[q_i; P[:,cols_i]] K-tiled (33 tiles, tile0 = q row),
    rhs=[wt; wx] K-tiled.  P symmetric so P[:,cols].T = P[cols,:].
  - G1A (fp8 e3m4): lhsT = SA*A[maskrows, cols_i] K-tiled (compacted),
    rhs = SW*wy[maskrows] K-tiled.  o1 = psP + psA/(SA*SW).
  - G2 (fp8): lhsT = -SA*A[rows_i,:].T K-tiled (32 tiles),
    rhs = SW*wx (cast on device from the bf16 W stream). b wt added via a
    tiny bf16 contraction-1 matmul with lhsT = b*SA*SW.
    o2 = ps2/(SA*SW) + (1-mask)*wy.  Optional e4m3+DoubleRow mode.
  - o3 partial per core: cf@wx_i + (-b_i)@(mask*wy_i); host adds xTPx*wt.
  PSUM: 2 banks G1P + 2 banks G1A + 4 banks G2 (2 accumulators per bank,
  bank-shared start/stop flags).

All streamed operands staged in DRAM K-tile-transposed: (128, ktiles*free)
with element (p, k*free+c) = orig(k*128+p, c).
"""

import numpy as np
import ml_dtypes
from contextlib import ExitStack

BF = ml_dtypes.bfloat16
E3 = ml_dtypes.float8_e3m4
E4 = ml_dtypes.float8_e4m3

N, M, KP = 4096, 8192, 256
NC = 8
NS, MS = N // NC, M // NC          # 512, 1024
KTP = 33                           # G1P k-tiles: [q/wt tile] + 32 P tiles
KT2 = 32                           # G2 k-tiles (full n contraction)

G2_DR = False                      # G2 as e4m3 + DoubleRow (else e3m4 normal)

_NC_CACHE = {}


def _kt(a, ktiles, free):
    """(ktiles*128, free) row-major -> (128, ktiles*free) K-tile-transposed."""
    return np.ascontiguousarray(
        a.reshape(ktiles, 128, free).transpose(1, 0, 2).reshape(128, ktiles * free))


def _build_nc(kta, g2_dr, c1, c2):
    from concourse import bacc, tile, mybir
    from concourse.alu_op_type import AluOpType as op

    dtb = mybir.dt.bfloat16
    dtf = mybir.dt.float32
    dt8a = mybir.dt.float8e3
    dt8c = mybir.dt.float8e4 if g2_dr else mybir.dt.float8e3
    pm = mybir.MatmulPerfMode.DoubleRow if g2_dr else None

    nc = bacc.Bacc("TRN2", target_bir_lowering=False, debug=False)

    def din(name, shape, dt):
        return nc.dram_tensor(name, list(shape), dt, kind="ExternalInput").ap()

    pt = din("pt", (128, KTP * NS), dtb)      # [q; P cols] K-tiled
    wtb = din("wtb", (128, KTP * KP), dtb)    # [wt; wx] K-tiled
    at8 = din("at8", (128, kta * NS), dt8a)   # compacted SA*A rows, K-tiled
    dy8 = din("dy8", (128, kta * KP), dt8a)   # compacted SW*wy, K-tiled
    if g2_dr:
        ct8 = din("ct8", (128, KT2, MS), dt8c)
    else:
        ct8 = din("ct8", (128, KT2 * MS), dt8c)
    wx8d = din("wx8", (128, KT2, KP), dt8c)   # SW*wx, K-tiled (G2 rhs)
    bxd = din("bx", (1, MS), dtb)             # b_i * SA2 * SW
    cfd = din("cfb", (128, 4), dtb)           # cf shard, (4,128).T layout
    nbd = din("nbb", (128, 8), dtb)           # -b_i, (8,128).T layout
    ytd = din("yto", (128, 8), dtf)
    std = din("sto", (128, 8), dtf)
    wod = din("wosb", (128, 8 * KP), dtb)     # own wy rows K-tiled
    xwd = din("xw", (128, 4 * KP), dtb)       # own wx rows K-tiled
    out1 = nc.dram_tensor("out1", [128, 4 * KP], dtb, kind="ExternalOutput").ap()
    out2 = nc.dram_tensor("out2", [128, 8 * KP], dtb, kind="ExternalOutput").ap()
    out3 = nc.dram_tensor("out3", [1, KP], dtf, kind="ExternalOutput").ap()

    NSTEP = KTP + kta
    # group boundaries (k-tiles)
    WG = [0, 1, 3, 7, 15, 24, 33]
    AG = [0, 8, 16, 24, kta] if kta > 24 else [0, 8, 16, kta]
    AG = sorted(set(min(b, kta) for b in AG))
    YG = sorted(set(min(b, kta) for b in [0, 16, kta]))
    CG = [0, 4, 12, 20, 28, 32]

    def g_of(bounds):
        m = {}
        for g in range(len(bounds) - 1):
            for k in range(bounds[g], bounds[g + 1]):
                m[k] = g
        return m

    wg_of, ag_of, yg_of, cg_of = g_of(WG), g_of(AG), g_of(YG), g_of(CG)

    # G2 tick schedule: e3 -> 32 single-ktile ticks; DR -> 16 pair ticks
    nticks = KT2 // 2 if g2_dr else KT2
    last_tick_step = NSTEP - 10
    tick_step = [2 + round(t * (last_tick_step - 2) / (nticks - 1))
                 for t in range(nticks)]
    t2s = {}
    for t, s_ in enumerate(tick_step):
        t2s.setdefault(s_, []).append(t)
    # ct group g prefetch step: 5 steps before its first tick
    ct_load_step = {}
    for g in range(len(CG) - 1):
        first_tick = CG[g] // (2 if g2_dr else 1)
        ct_load_step.setdefault(max(0, tick_step[min(first_tick, nticks - 1)] - 5),
                                []).append(g)

    with tile.TileContext(nc) as tc, ExitStack() as ctx:
        dpool = ctx.enter_context(tc.tile_pool(name="d", bufs=1))
        wpool = ctx.enter_context(tc.tile_pool(name="w", bufs=3))
        ppool = ctx.enter_context(tc.tile_pool(name="p", bufs=3))
        apool = ctx.enter_context(tc.tile_pool(name="a", bufs=2))
        ypool = ctx.enter_context(tc.tile_pool(name="y", bufs=2))
        cpool = ctx.enter_context(tc.tile_pool(name="c", bufs=2))
        opool = ctx.enter_context(tc.tile_pool(name="o", bufs=1))
        pspool = ctx.enter_context(tc.tile_pool(name="ps", bufs=8, space="PSUM"))

        # PSUM: 8 banks, each (128, 512) fp32 holding two 256-wide slots
        psP = [pspool.tile((128, 2 * KP), dtf, tag="ps", name=f"psP{i}") for i in range(2)]
        psA = [pspool.tile((128, 2 * KP), dtf, tag="ps", name=f"psA{i}") for i in range(2)]
        ps2 = [pspool.tile((128, 2 * KP), dtf, tag="ps", name=f"ps2{i}") for i in range(4)]

        def pslot(tiles, t):
            return tiles[t // 2][:, (t % 2) * KP:(t % 2 + 1) * KP]

        # streaming group tiles
        wtbg, ptg, atg, dyg, ctg = {}, {}, {}, {}, {}

        def load_w(g):
            k0, k1 = WG[g], WG[g + 1]
            t = wpool.tile((128, (k1 - k0) * KP), dtb, tag="wtb", name=f"wtbg{g}",
                           padded_shape=(128, 9 * KP))
            nc.scalar.dma_start(t, wtb[:, k0 * KP:k1 * KP])
            wtbg[g] = t

        def load_p(g):
            k0, k1 = WG[g], WG[g + 1]
            t = ppool.tile((128, (k1 - k0) * NS), dtb, tag="pt", name=f"ptg{g}",
                           padded_shape=(128, 9 * NS))
            nc.sync.dma_start(t, pt[:, k0 * NS:k1 * NS])
            ptg[g] = t

        def load_a(g):
            k0, k1 = AG[g], AG[g + 1]
            t = apool.tile((128, (k1 - k0) * NS), dt8a, tag="at",
                           name=f"atg{g}", padded_shape=(128, 9 * NS))
            nc.sync.dma_start(t, at8[:, k0 * NS:k1 * NS])
            atg[g] = t

        def load_y(g):
            k0, k1 = YG[g], YG[g + 1]
            t = ypool.tile((128, (k1 - k0) * KP), dt8a, tag="dy", name=f"dyg{g}",
                           padded_shape=(128, 17 * KP))
            nc.gpsimd.dma_start(t, dy8[:, k0 * KP:k1 * KP])
            dyg[g] = t

        def load_c(g):
            j0, j1 = CG[g], CG[g + 1]
            if g2_dr:
                t = cpool.tile((128, 8, MS), dt8c, tag="ct", name=f"ctg{g}")
                nc.gpsimd.dma_start(t[:, 0:j1 - j0, :], ct8[:, j0:j1, :])
            else:
                t = cpool.tile((128, 8 * MS), dt8c, tag="ct", name=f"ctg{g}",
                               padded_shape=(128, 8 * MS))
                nc.gpsimd.dma_start(t[:, :(j1 - j0) * MS], ct8[:, j0 * MS:j1 * MS])
            ctg[g] = t

        # wx8: resident fp8 G2 rhs, host-prepared, loaded in 2 chunks
        wx8a = dpool.tile((128, 8, KP), dt8c, tag="wx8a", name="wx8a")
        wx8b = dpool.tile((128, KT2 - 8, KP), dt8c, tag="wx8b", name="wx8b")
        nc.gpsimd.dma_start(wx8a, wx8d[:, 0:8, :])

        def wx8_rhs(j):
            return wx8a[:, j, :] if j < 8 else wx8b[:, j - 8, :]

        def wx8_rhs_pair(p_):
            return (wx8a[:, 2 * p_:2 * p_ + 2, 0:KP] if p_ < 4
                    else wx8b[:, 2 * p_ - 8:2 * p_ - 6, 0:KP])

        sm = {}

        def emit_smalls():
            for nm, dref, shp, dt in (("bxs", bxd, (1, MS), dtb),
                                      ("cfs", cfd, (128, 4), dtb),
                                      ("nbs", nbd, (128, 8), dtb),
                                      ("yts", ytd, (128, 8), dtf),
                                      ("sts", std, (128, 8), dtf)):
                t = dpool.tile(shp, dt, tag=nm, name=nm)
                nc.scalar.dma_start(t, dref)
                sm[nm] = t
            t = dpool.tile((128, 8 * KP), dtb, tag="wos", name="wos")
            nc.scalar.dma_start(t, wod)
            sm["wos"] = t
            t = dpool.tile((128, 4 * KP), dtb, tag="xws", name="xws")
            nc.scalar.dma_start(t, xwd)
            sm["xws"] = t

        def emit_masks():
            vo = dpool.tile((128, 8), dtf, tag="vo", name="vo")
            nc.vector.tensor_sub(vo, sm["yts"], sm["sts"])
            masko = dpool.tile((128, 8), dtf, tag="masko", name="masko")
            nc.vector.tensor_scalar(masko, vo, 0.0, None, op.is_ge)
            umo = dpool.tile((128, 8), dtf, tag="umo", name="umo")
            nc.vector.tensor_scalar(umo, masko, -1.0, 1.0, op.mult, op.add)
            sm["umo"] = umo

        def emit_wom():
            wom = dpool.tile((128, 8 * KP), dtb, tag="wom", name="wom")
            wmt = dpool.tile((128, 8 * KP), dtb, tag="wmt", name="wmt")
            for t_i in range(8):
                sl = slice(t_i * KP, (t_i + 1) * KP)
                nc.vector.tensor_scalar_mul(wom[:, sl], sm["wos"][:, sl],
                                            sm["umo"][:, t_i:t_i + 1])
            for t_i in range(8):
                sl = slice(t_i * KP, (t_i + 1) * KP)
                nc.vector.tensor_sub(wmt[:, sl], sm["wos"][:, sl], wom[:, sl])
            sm["wom"] = wom
            sm["wmt"] = wmt

        prp = dpool.tile((128, 4 * KP), dtf, tag="prp", name="prp")
        ob1 = opool.tile((128, 4 * KP), dtb, tag="ob1", name="ob1")
        ob2 = opool.tile((128, 8 * KP), dtb, tag="ob2", name="ob2")

        load_w(0)
        load_p(0)
        load_w(1)
        load_p(1)

        done_ticks = 0
        at_started = False
        for k in range(NSTEP):
            is_p = k < KTP
            kk = k if is_p else k - KTP

            # --- JIT stream prefetch ---
            if is_p:
                g = wg_of[kk]
                if kk == WG[g] and g + 2 <= len(WG) - 2:
                    load_w(g + 2)
                    load_p(g + 2)
            else:
                g = ag_of[kk]
                if kk == AG[g] and g + 1 <= len(AG) - 2:
                    load_a(g + 1)
                if kk == YG[yg_of[kk]] and yg_of[kk] + 1 <= len(YG) - 2:
                    load_y(yg_of[kk] + 1)
            if k == KTP - 8:
                load_a(0)
                load_y(0)
                at_started = True
            for g in ct_load_step.get(k, []):
                load_c(g)

            # --- G1 matmuls (4 m-blocks into 2 shared banks) ---
            if is_p:
                g = wg_of[kk]
                rhs = wtbg[g][:, (kk - WG[g]) * KP:(kk - WG[g] + 1) * KP]
                pg = wg_of[kk]
                lt = ptg[pg]
                jo = kk - WG[pg]
                for m in range(4):
                    nc.tensor.matmul(
                        pslot(psP, m),
                        lt[:, jo * NS + m * 128:jo * NS + (m + 1) * 128],
                        rhs, start=(kk == 0 and m % 2 == 0),
                        stop=(kk == KTP - 1 and m % 2 == 1))
            else:
                g = ag_of[kk]
                yg = yg_of[kk]
                rhs = dyg[yg][:, (kk - YG[yg]) * KP:(kk - YG[yg] + 1) * KP]
                lt = atg[g]
                jo = kk - AG[g]
                for m in range(4):
                    nc.tensor.matmul(
                        pslot(psA, m),
                        lt[:, jo * NS + m * 128:jo * NS + (m + 1) * 128],
                        rhs, start=(kk == 0 and m % 2 == 0),
                        stop=(kk == kta - 1 and m % 2 == 1))

            # --- b (x) wt into ps2 (owns the start flags) ---
            if k == 1:
                for t_i in range(8):
                    nc.tensor.matmul(
                        pslot(ps2, t_i),
                        sm["bxs"][0:1, t_i * 128:(t_i + 1) * 128],
                        wtbg[0][0:1, 0:KP],
                        start=(t_i % 2 == 0), stop=False)

            # --- second wx8 chunk ---
            if k == 2:
                nc.gpsimd.dma_start(wx8b, wx8d[:, 8:KT2, :])

            # --- G2 ticks ---
            for t in t2s.get(k, []):
                if g2_dr:
                    pair = t
                    g = cg_of[2 * pair]
                    jo = 2 * pair - CG[g]
                    for t_i in range(8):
                        nc.tensor.matmul(
                            pslot(ps2, t_i),
                            ctg[g][:, jo:jo + 2, t_i * 128:(t_i + 1) * 128],
                            wx8_rhs_pair(pair),
                            start=False,
                            stop=(pair == KT2 // 2 - 1 and t_i % 2 == 1),
                            perf_mode=pm)
                else:
                    j = t
                    g = cg_of[j]
                    jo = j - CG[g]
                    for t_i in range(8):
                        nc.tensor.matmul(
                            pslot(ps2, t_i),
                            ctg[g][:, jo * MS + t_i * 128:jo * MS + (t_i + 1) * 128],
                            wx8_rhs(j),
                            start=False,
                            stop=(j == KT2 - 1 and t_i % 2 == 1))
                done_ticks += 1

            # --- deferred small loads + mask computation ---
            if k == 0:
                emit_smalls()
            if k == 8:
                emit_masks()
            if k == 26:
                emit_wom()

            # --- evict psP to SBUF right after G1P finishes ---
            if k == KTP:
                for m in range(4):
                    nc.vector.tensor_copy(prp[:, m * KP:(m + 1) * KP], pslot(psP, m))

            # --- o2 eviction once G2 is done ---
            if done_ticks == nticks:
                done_ticks = -1  # fire once
                for t_i in range(8):
                    sl = slice(t_i * KP, (t_i + 1) * KP)
                    nc.vector.scalar_tensor_tensor(
                        ob2[:, sl], pslot(ps2, t_i), c2, sm["wom"][:, sl],
                        op.mult, op.add)
                nc.scalar.dma_start(out2, ob2)
                # o3 partial: (-b)@dy + cf@wx over own shards
                pso3 = pspool.tile((1, KP), dtf, tag="ps", name="pso3")
                for t_i in range(8):
                    nc.tensor.matmul(pso3, sm["nbs"][:, t_i:t_i + 1],
                                     sm["wmt"][:, t_i * KP:(t_i + 1) * KP],
                                     start=(t_i == 0), stop=False)
                for j3 in range(4):
                    nc.tensor.matmul(pso3, sm["cfs"][:, j3:j3 + 1],
                                     sm["xws"][:, j3 * KP:(j3 + 1) * KP],
                                     start=False, stop=(j3 == 3))
                o3f = opool.tile((1, KP), dtf, tag="o3f", name="o3f")
                nc.vector.tensor_copy(o3f, pso3)
                nc.scalar.dma_start(out3, o3f)

        # --- final o1 combine: psP (via prp) + psA/(SA*SW) ---
        for m in range(4):
            nc.vector.scalar_tensor_tensor(
                ob1[:, m * KP:(m + 1) * KP], pslot(psA, m), c1,
                prp[:, m * KP:(m + 1) * KP], op.mult, op.add)
        nc.scalar.dma_start(out1, ob1)

    nc.compile()
    return nc


SW_DEV = 1.0  # device-side wx scale (set by _prep before build)


def _pow2_scale(std, mx, limit):
    if not np.isfinite(std) or std <= 0:
        return 1.0
    s = 2.0 ** round(np.log2(1.0 / std))
    while mx * s > limit:
        s *= 0.5
    return s


def _get_nc(key):
    if key not in _NC_CACHE:
        _NC_CACHE[key] = _build_nc(*key)
    return _NC_CACHE[key]


def _prep(P, A, q, b, x, y, s, W):
    global SW_DEV
    P = np.asarray(P, np.float32)
    A = np.asarray(A, np.float32)
    q = np.asarray(q, np.float32)
    b = np.asarray(b, np.float32)
    x = np.asarray(x, np.float32)
    y = np.asarray(y, np.float32)
    s = np.asarray(s, np.float32)
    W = np.asarray(W, np.float32)

    mb = (y - s) >= 0
    idx = np.nonzero(mb)[0]
    mp = max(1, len(idx))
    kta = (mp + 127) // 128

    wx, wy, wt = W[:N], W[N:N + M], W[N + M:]
    e4 = G2_DR
    SA = _pow2_scale(A.std(), np.abs(A).max(), 200.0 if e4 else 14.0)
    SW = _pow2_scale(1.0, np.abs(W).max(), 200.0 if e4 else 14.0)
    SW_DEV = SW
    c1 = 1.0 / (SA * SW)
    c2 = 1.0 / (SA * SW)

    Px = P @ x
    xPx = float(x @ Px)
    cf = -(q + 2.0 * Px)

    # shared (core-independent) staging
    wtb_full = np.zeros((KTP * 128, KP), BF)
    wtb_full[0] = wt[0]
    wtb_full[128:128 + N] = wx.astype(BF)
    wtb_h = _kt(wtb_full, KTP, KP)

    Ac = A[idx]                       # (mp, N)
    at_q = (Ac * SA).astype(E3)       # quantize once, slice per core
    dy_full = np.zeros((kta * 128, KP), E3)
    dy_full[:mp] = (wy[idx] * SW).astype(E3)
    dy_h = _kt(dy_full, kta, KP)

    E4c = E4 if e4 else E3
    wx8_h = _kt((wx * SW).astype(E4c), KT2, KP).reshape(128, KT2, KP)
    in_maps = []
    for i in range(NC):
        ncol = slice(i * NS, (i + 1) * NS)
        mrow = slice(i * MS, (i + 1) * MS)
        pt0 = np.zeros((KTP * 128, NS), BF)
        pt0[0] = q[ncol].astype(BF)
        pt0[128:128 + N] = P[:, ncol].astype(BF)
        at0 = np.zeros((kta * 128, NS), E3)
        at0[:mp] = at_q[:, ncol]
        ct0 = (-(SA * A[mrow].T)).astype(E4c)          # (N, MS)
        ct_h = _kt(ct0, KT2, MS)
        if G2_DR:
            ct_h = ct_h.reshape(128, KT2, MS)
        yto = np.ascontiguousarray(y[mrow].reshape(8, 128).T)
        sto = np.ascontiguousarray(s[mrow].reshape(8, 128).T)
        in_maps.append(dict(
            pt=_kt(pt0, KTP, NS), wtb=wtb_h,
            at8=_kt(at0, kta, NS), dy8=dy_h, ct8=ct_h, wx8=wx8_h,
            bx=np.ascontiguousarray((b[mrow] * SA * SW)[None, :].astype(BF)),
            cfb=np.ascontiguousarray(cf[ncol].reshape(4, 128).T.astype(BF)),
            nbb=np.ascontiguousarray((-b[mrow]).reshape(8, 128).T.astype(BF)),
            yto=yto, sto=sto,
            wosb=_kt(wy[mrow].astype(BF), 8, KP),
            xw=_kt(wx[ncol].astype(BF), 4, KP),
        ))
    return in_maps, kta, c1, c2, xPx, wt


def _assemble(results, xPx, wt):
    Fo = np.empty((N + M + 1, KP), np.float32)
    o3 = xPx * wt[0].astype(np.float32)
    for i in range(NC):
        o1 = np.asarray(results[i]["out1"], np.float32)     # (128, 4*KP)
        o2 = np.asarray(results[i]["out2"], np.float32)     # (128, 8*KP)
        Fo[i * NS:(i + 1) * NS] = (
            o1.reshape(128, 4, KP).transpose(1, 0, 2).reshape(NS, KP))
        Fo[N + i * MS:N + (i + 1) * MS] = (
            o2.reshape(128, 8, KP).transpose(1, 0, 2).reshape(MS, KP))
        o3 = o3 + np.asarray(results[i]["out3"], np.float32)[0]
    Fo[N + M] = o3
    return Fo


def _run_sharded(inputs, trace=False, trace_kwargs=None):
    from concourse import bass_utils
    in_maps, kta, c1, c2, xPx, wt = _prep(**inputs)
    nc = _get_nc((kta, G2_DR, c1, c2))
    res = bass_utils.run_bass_kernel_spmd(
        nc, in_maps, core_ids=list(range(NC)), trace=trace,
        **(trace_kwargs or {}))
    return _assemble(res.results, xPx, wt), res


def kernel(**inputs) -> np.ndarray:
    out, _ = _run_sharded(inputs, trace=False)
    return out


# revision 16
# speedup vs baseline: 1.3533x; 1.2463x over previous
"""Trainium2 Bass kernel for the AbstractQCP residual operator F @ W.

Math (reference):
    v = y - s; mask = (v >= 0)
    dx = wx; dy = mask*wy; dt = wt        (W = [wx; wy; wt], (n+m+1, K))
    o1 = P@wx + A.T@dy + q wt             (n, K)
    o2 = b wt - A@wx                      (m, K)
    o3 = (x.T P x) wt - (q + 2 P x)@wx - b@dy
    F  = [o1; o2 + (1-mask)*wy; o3]

Design (per core i of 8, pure SPMD, host gathers):
  core i owns o1 rows [512i,512(i+1)) and o2 rows [1024i,1024(i+1)).
  Host precomputes: mask, row-compacted A.T@dy operands (only rows with
  mask=1 contribute), Px = P@x, xTPx, cf = -(q+2Px).
  All big operands fp8 E3M4 scaled by a single power-of-two S (=64):
    G1P: lhsT = S*P[:,cols_i] (32 ktiles; P symmetric), rhs = wx8 = wx
    G1A: lhsT = S*A[maskrows, cols_i] (compacted ktiles), rhs = dy8
    q x) wt: contraction-1 bf16 matmul, lhsT = S*q_i, rhs = wt row
    --> all accumulate in ONE psum set (identical scale); o1 = ps1/S.
    G2:  lhsT = -S*A[rows_i,:].T (32 ktiles), rhs = wx8 (shared tiles!);
         b wt via contraction-1 bf16 matmul lhsT = S*b_i.
         o2 = ps2/S + (1-mask)*wy.  Optional e4m3+DoubleRow mode.
  o3 partial per core: cf@wx_i + (-b_i)@(mask*wy_i); host adds xTPx*wt.
  PSUM: 2 banks o1 + 4 banks G2 (2 x 256-wide accumulators per bank,
  bank-shared start/stop flags) + 1 bank o3.
  DMA: ~11.6 MB/core balanced over the 3 trigger queues (sync/scalar/
  gpsimd), ~0.5-1 MB per transfer.

Streamed operands staged in DRAM K-tile-transposed: (128, ktiles*free)
with element (p, k*free+c) = orig(k*128+p, c).
"""

import numpy as np
import ml_dtypes
from contextlib import ExitStack

BF = ml_dtypes.bfloat16
E3 = ml_dtypes.float8_e3m4
E4 = ml_dtypes.float8_e4m3

N, M, KP = 4096, 8192, 256
NC = 8
NS, MS = N // NC, M // NC          # 512, 1024
KTP = 32                           # P k-tiles
KT2 = 32                           # G2 k-tiles (full n contraction)

G2_DR = False                      # G2 as e4m3 + DoubleRow (else e3m4)

_NC_CACHE = {}


def _kt(a, ktiles, free):
    """(ktiles*128, free) row-major -> (128, ktiles*free) K-tile-transposed."""
    return np.ascontiguousarray(
        a.reshape(ktiles, 128, free).transpose(1, 0, 2).reshape(128, ktiles * free))


def _build_nc(kta, g2_dr, c_inv):
    from concourse import bacc, tile, mybir
    from concourse.alu_op_type import AluOpType as op

    dtb = mybir.dt.bfloat16
    dtf = mybir.dt.float32
    dt8 = mybir.dt.float8e3
    dt8c = mybir.dt.float8e4 if g2_dr else mybir.dt.float8e3
    pm = mybir.MatmulPerfMode.DoubleRow if g2_dr else None

    nc = bacc.Bacc("TRN2", target_bir_lowering=False, debug=False)

    def din(name, shape, dt):
        return nc.dram_tensor(name, list(shape), dt, kind="ExternalInput").ap()

    pt8 = din("pt8", (128, KTP * NS), dt8)    # S*P[:,cols] K-tiled
    at8 = din("at8", (128, kta * NS), dt8)    # compacted S*A rows, K-tiled
    dy8 = din("dy8", (128, kta * KP), dt8)    # compacted wy, K-tiled
    ct8 = din("ct8", (128, KT2, MS), dt8c)    # -S*A[rows].T K-tiled
    wx8 = din("wx8", (128, KT2, KP), dt8c)    # wx K-tiled (G1P + G2 rhs)
    wtr = din("wtr", (1, KP), dtb)            # wt row
    qxd = din("qx", (1, NS), dtb)             # S*q_i
    bxd = din("bx", (1, MS), dtb)             # S*b_i
    cfd = din("cfb", (128, 4), dtb)           # cf shard, (4,128).T layout
    nbd = din("nbb", (128, 8), dtb)           # -b_i, (8,128).T layout
    ytd = din("yto", (128, 8), dtf)
    std = din("sto", (128, 8), dtf)
    wod = din("wosb", (128, 8 * KP), dtb)     # own wy rows K-tiled
    xwd = din("xw", (128, 4 * KP), dtb)       # own wx rows K-tiled
    out1 = nc.dram_tensor("out1", [128, 4 * KP], dtb, kind="ExternalOutput").ap()
    out2 = nc.dram_tensor("out2", [128, 8 * KP], dtb, kind="ExternalOutput").ap()
    out3 = nc.dram_tensor("out3", [1, KP], dtf, kind="ExternalOutput").ap()

    NSTEP = KTP + kta
    PG = [0, 1, 3, 7, 15, 24, 32]             # pt8 groups (sync)
    AG = sorted(set(min(b, kta) for b in [0, 8, 16, 24, kta]))  # at8 (scalar)
    YG = sorted(set(min(b, kta) for b in [0, 16, kta]))         # dy8 (sync)
    CG = [0, 8, 16, 24, 32]                   # ct8 groups (gpsimd)
    WXG = [0, 4, 16, 32]                      # wx8 chunks (scalar)

    def g_of(bounds):
        m = {}
        for g in range(len(bounds) - 1):
            for k in range(bounds[g], bounds[g + 1]):
                m[k] = g
        return m

    pg_of, ag_of, yg_of, cg_of = g_of(PG), g_of(AG), g_of(YG), g_of(CG)

    nticks = KT2 // 2 if g2_dr else KT2
    first_tick, last_tick = 5, NSTEP - 10
    tick_step = [first_tick + round(t * (last_tick - first_tick) / (nticks - 1))
                 for t in range(nticks)]
    t2s = {}
    for t, s_ in enumerate(tick_step):
        t2s.setdefault(s_, []).append(t)
    ct_load_step = {}
    for g in range(len(CG) - 1):
        ft = CG[g] // (2 if g2_dr else 1)
        ct_load_step.setdefault(max(0, tick_step[min(ft, nticks - 1)] - 6),
                                []).append(g)

    with tile.TileContext(nc) as tc, ExitStack() as ctx:
        dpool = ctx.enter_context(tc.tile_pool(name="d", bufs=1))
        ppool = ctx.enter_context(tc.tile_pool(name="p", bufs=3))
        apool = ctx.enter_context(tc.tile_pool(name="a", bufs=2))
        ypool = ctx.enter_context(tc.tile_pool(name="y", bufs=2))
        cpool = ctx.enter_context(tc.tile_pool(name="c", bufs=2))
        opool = ctx.enter_context(tc.tile_pool(name="o", bufs=1))
        pspool = ctx.enter_context(tc.tile_pool(name="ps", bufs=8, space="PSUM"))

        ps1 = [pspool.tile((128, 2 * KP), dtf, tag="ps", name=f"ps1{i}") for i in range(2)]
        ps2 = [pspool.tile((128, 2 * KP), dtf, tag="ps", name=f"ps2{i}") for i in range(4)]

        def pslot(tiles, t):
            return tiles[t // 2][:, (t % 2) * KP:(t % 2 + 1) * KP]

        ptg, atg, dyg, ctg = {}, {}, {}, {}

        def load_p(g):
            k0, k1 = PG[g], PG[g + 1]
            t = ppool.tile((128, (k1 - k0) * NS), dt8, tag="pt", name=f"ptg{g}",
                           padded_shape=(128, 9 * NS))
            nc.sync.dma_start(t, pt8[:, k0 * NS:k1 * NS])
            ptg[g] = t

        def load_a(g):
            k0, k1 = AG[g], AG[g + 1]
            t = apool.tile((128, (k1 - k0) * NS), dt8, tag="at",
                           name=f"atg{g}", padded_shape=(128, 9 * NS))
            nc.scalar.dma_start(t, at8[:, k0 * NS:k1 * NS])
            atg[g] = t

        def load_y(g):
            k0, k1 = YG[g], YG[g + 1]
            t = ypool.tile((128, (k1 - k0) * KP), dt8, tag="dy", name=f"dyg{g}",
                           padded_shape=(128, 17 * KP))
            nc.sync.dma_start(t, dy8[:, k0 * KP:k1 * KP])
            dyg[g] = t

        def load_c(g):
            j0, j1 = CG[g], CG[g + 1]
            t = cpool.tile((128, 8, MS), dt8c, tag="ct", name=f"ctg{g}")
            nc.gpsimd.dma_start(t[:, 0:j1 - j0, :], ct8[:, j0:j1, :])
            ctg[g] = t

        # wx8 resident chunks (scalar queue, front-loaded)
        wxc = []
        for ci, (j0, j1) in enumerate(zip(WXG[:-1], WXG[1:])):
            t = dpool.tile((128, j1 - j0, KP), dt8c, tag=f"wxc{ci}", name=f"wxc{ci}")
            wxc.append(t)

        def wx8_rhs(j):
            ci = 0 if j < 4 else (1 if j < 16 else 2)
            return wxc[ci][:, j - WXG[ci], :]

        def wx8_rhs_pair(p_):
            j = 2 * p_
            ci = 0 if j < 4 else (1 if j < 16 else 2)
            return wxc[ci][:, j - WXG[ci]:j - WXG[ci] + 2, 0:KP]

        sm = {}

        def emit_first_smalls():
            for nm, dref, shp, dt in (("qxs", qxd, (1, NS), dtb),
                                      ("wts", wtr, (1, KP), dtb),
                                      ("bxs", bxd, (1, MS), dtb)):
                t = dpool.tile(shp, dt, tag=nm, name=nm)
                nc.scalar.dma_start(t, dref)
                sm[nm] = t

        def emit_smalls():
            for nm, dref, shp, dt in (("cfs", cfd, (128, 4), dtb),
                                      ("nbs", nbd, (128, 8), dtb),
                                      ("yts", ytd, (128, 8), dtf),
                                      ("sts", std, (128, 8), dtf)):
                t = dpool.tile(shp, dt, tag=nm, name=nm)
                nc.scalar.dma_start(t, dref)
                sm[nm] = t

        def emit_masks():
            vo = dpool.tile((128, 8), dtf, tag="vo", name="vo")
            nc.vector.tensor_sub(vo, sm["yts"], sm["sts"])
            masko = dpool.tile((128, 8), dtf, tag="masko", name="masko")
            nc.vector.tensor_scalar(masko, vo, 0.0, None, op.is_ge)
            umo = dpool.tile((128, 8), dtf, tag="umo", name="umo")
            nc.vector.tensor_scalar(umo, masko, -1.0, 1.0, op.mult, op.add)
            sm["umo"] = umo

        def emit_wom():
            wom = dpool.tile((128, 8 * KP), dtb, tag="wom", name="wom")
            wmt = dpool.tile((128, 8 * KP), dtb, tag="wmt", name="wmt")
            for t_i in range(8):
                sl = slice(t_i * KP, (t_i + 1) * KP)
                nc.vector.tensor_scalar_mul(wom[:, sl], sm["wos"][:, sl],
                                            sm["umo"][:, t_i:t_i + 1])
            for t_i in range(8):
                sl = slice(t_i * KP, (t_i + 1) * KP)
                nc.vector.tensor_sub(wmt[:, sl], sm["wos"][:, sl], wom[:, sl])
            sm["wom"] = wom
            sm["wmt"] = wmt

        ob1 = opool.tile((128, 4 * KP), dtb, tag="ob1", name="ob1")
        ob2 = opool.tile((128, 8 * KP), dtb, tag="ob2", name="ob2")

        # front-loaded triggers: smalls + wx8 c0/c1 on scalar, pt g0/g1 on sync
        emit_first_smalls()
        nc.scalar.dma_start(wxc[0], wx8[:, WXG[0]:WXG[1], :])
        load_p(0)
        nc.scalar.dma_start(wxc[1], wx8[:, WXG[1]:WXG[2], :])
        load_p(1)

        done_ticks = 0
        for k in range(NSTEP):
            is_p = k < KTP
            kk = k if is_p else k - KTP

            # --- JIT stream prefetch ---
            if is_p:
                g = pg_of[kk]
                if kk == PG[g] and g + 2 <= len(PG) - 2:
                    load_p(g + 2)
            else:
                g = ag_of[kk]
                if kk == AG[g] and g + 1 <= len(AG) - 2:
                    load_a(g + 1)
                yg = yg_of[kk]
                if kk == YG[yg] and yg + 1 <= len(YG) - 2:
                    load_y(yg + 1)
            if k == 2:
                nc.scalar.dma_start(wxc[2], wx8[:, WXG[2]:WXG[3], :])
            if k == 10:
                emit_smalls()
            if k == 14:
                emit_masks()
            if k == KTP - 12:
                load_a(0)
            if k == KTP - 8:
                load_y(0)
            if k == KTP + 2:
                t = dpool.tile((128, 8 * KP), dtb, tag="wos", name="wos")
                nc.sync.dma_start(t, wod)
                sm["wos"] = t
                t = dpool.tile((128, 4 * KP), dtb, tag="xws", name="xws")
                nc.sync.dma_start(t, xwd)
                sm["xws"] = t
            if k == KTP + 6:
                emit_wom()
            for g in ct_load_step.get(k, []):
                load_c(g)

            # --- q (x) wt opens the ps1 accumulation group ---
            if k == 0:
                for m in range(4):
                    nc.tensor.matmul(
                        pslot(ps1, m), sm["qxs"][0:1, m * 128:(m + 1) * 128],
                        sm["wts"][0:1, 0:KP],
                        start=(m % 2 == 0), stop=False)

            # --- G1 matmuls (4 m-blocks into 2 shared banks) ---
            if is_p:
                g = pg_of[kk]
                rhs = wx8_rhs(kk)
                lt = ptg[g]
                jo = kk - PG[g]
            else:
                g = ag_of[kk]
                yg = yg_of[kk]
                rhs = dyg[yg][:, (kk - YG[yg]) * KP:(kk - YG[yg] + 1) * KP]
                lt = atg[g]
                jo = kk - AG[g]
            for m in range(4):
                nc.tensor.matmul(
                    pslot(ps1, m),
                    lt[:, jo * NS + m * 128:jo * NS + (m + 1) * 128],
                    rhs, start=False,
                    stop=(not is_p and kk == kta - 1 and m % 2 == 1))

            # --- b (x) wt opens the ps2 accumulation group ---
            if k == 1:
                for t_i in range(8):
                    nc.tensor.matmul(
                        pslot(ps2, t_i),
                        sm["bxs"][0:1, t_i * 128:(t_i + 1) * 128],
                        sm["wts"][0:1, 0:KP],
                        start=(t_i % 2 == 0), stop=False)

            # --- G2 ticks ---
            for t in t2s.get(k, []):
                if g2_dr:
                    pair = t
                    g = cg_of[2 * pair]
                    jo = 2 * pair - CG[g]
                    for t_i in range(8):
                        nc.tensor.matmul(
                            pslot(ps2, t_i),
                            ctg[g][:, jo:jo + 2, t_i * 128:(t_i + 1) * 128],
                            wx8_rhs_pair(pair),
                            start=False,
                            stop=(pair == KT2 // 2 - 1 and t_i % 2 == 1),
                            perf_mode=pm)
                else:
                    j = t
                    g = cg_of[j]
                    jo = j - CG[g]
                    for t_i in range(8):
                        nc.tensor.matmul(
                            pslot(ps2, t_i),
                            ctg[g][:, jo, t_i * 128:(t_i + 1) * 128],
                            wx8_rhs(j),
                            start=False,
                            stop=(j == KT2 - 1 and t_i % 2 == 1))
                done_ticks += 1

            # --- o2 eviction + o3 once G2 is done ---
            if done_ticks == nticks:
                done_ticks = -1
                for t_i in range(8):
                    sl = slice(t_i * KP, (t_i + 1) * KP)
                    nc.vector.scalar_tensor_tensor(
                        ob2[:, sl], pslot(ps2, t_i), c_inv, sm["wom"][:, sl],
                        op.mult, op.add)
                nc.scalar.dma_start(out2, ob2)
                pso3 = pspool.tile((1, KP), dtf, tag="ps", name="pso3")
                for t_i in range(8):
                    nc.tensor.matmul(pso3, sm["nbs"][:, t_i:t_i + 1],
                                     sm["wmt"][:, t_i * KP:(t_i + 1) * KP],
                                     start=(t_i == 0), stop=False)
                for j3 in range(4):
                    nc.tensor.matmul(pso3, sm["cfs"][:, j3:j3 + 1],
                                     sm["xws"][:, j3 * KP:(j3 + 1) * KP],
                                     start=False, stop=(j3 == 3))
                o3f = opool.tile((1, KP), dtf, tag="o3f", name="o3f")
                nc.vector.tensor_copy(o3f, pso3)
                nc.scalar.dma_start(out3, o3f)

        # --- final o1 eviction: ob1 = ps1 / S ---
        for m in range(4):
            nc.vector.tensor_scalar_mul(ob1[:, m * KP:(m + 1) * KP],
                                        pslot(ps1, m), c_inv)
        nc.scalar.dma_start(out1, ob1)

    nc.compile()
    return nc


def _get_nc(key):
    if key not in _NC_CACHE:
        _NC_CACHE[key] = _build_nc(*key)
    return _NC_CACHE[key]


def _pow2_scale(std, mx, limit):
    if not np.isfinite(std) or std <= 0:
        return 1.0
    s = 2.0 ** round(np.log2(1.0 / std))
    while mx * s > limit:
        s *= 0.5
    return s


def _prep(P, A, q, b, x, y, s, W):
    P = np.asarray(P, np.float32)
    A = np.asarray(A, np.float32)
    q = np.asarray(q, np.float32)
    b = np.asarray(b, np.float32)
    x = np.asarray(x, np.float32)
    y = np.asarray(y, np.float32)
    s = np.asarray(s, np.float32)
    W = np.asarray(W, np.float32)

    mb = (y - s) >= 0
    idx = np.nonzero(mb)[0]
    mp = max(1, len(idx))
    kta = (mp + 127) // 128

    wx, wy, wt = W[:N], W[N:N + M], W[N + M:]
    SA = _pow2_scale(A.std(), np.abs(A).max(), 14.0)
    SW = _pow2_scale(1.0, np.abs(W).max(), 14.0)
    c_inv = 1.0 / (SA * SW)

    Px = P @ x
    xPx = float(x @ Px)
    cf = -(q + 2.0 * Px)

    E4c = E4 if G2_DR else E3
    wx8_h = _kt((wx * SW).astype(E4c), KT2, KP).reshape(128, KT2, KP)
    at_q = (A[idx] * SA).astype(E3)          # (mp, N), quantize once
    dy_full = np.zeros((kta * 128, KP), E3)
    dy_full[:mp] = (wy[idx] * SW).astype(E3)
    dy_h = _kt(dy_full, kta, KP)
    wtr_h = np.ascontiguousarray(wt.astype(BF))

    in_maps = []
    for i in range(NC):
        ncol = slice(i * NS, (i + 1) * NS)
        mrow = slice(i * MS, (i + 1) * MS)
        pt0 = (P[:, ncol] * SA).astype(E3)                   # (N, NS)
        at0 = np.zeros((kta * 128, NS), E3)
        at0[:mp] = at_q[:, ncol]
        ct0 = (-(SA * A[mrow].T)).astype(E4c)                # (N, MS)
        in_maps.append(dict(
            pt8=_kt(pt0, KTP, NS),
            at8=_kt(at0, kta, NS), dy8=dy_h,
            ct8=_kt(ct0, KT2, MS).reshape(128, KT2, MS), wx8=wx8_h,
            wtr=wtr_h,
            qx=np.ascontiguousarray((q[ncol] * SA * SW)[None, :].astype(BF)),
            bx=np.ascontiguousarray((b[mrow] * SA * SW)[None, :].astype(BF)),
            cfb=np.ascontiguousarray(cf[ncol].reshape(4, 128).T.astype(BF)),
            nbb=np.ascontiguousarray((-b[mrow]).reshape(8, 128).T.astype(BF)),
            yto=np.ascontiguousarray(y[mrow].reshape(8, 128).T),
            sto=np.ascontiguousarray(s[mrow].reshape(8, 128).T),
            wosb=_kt(wy[mrow].astype(BF), 8, KP),
            xw=_kt(wx[ncol].astype(BF), 4, KP),
        ))
    return in_maps, kta, c_inv, xPx, wt


def _assemble(results, xPx, wt):
    Fo = np.empty((N + M + 1, KP), np.float32)
    o3 = xPx * wt[0].astype(np.float32)
    for i in range(NC):
        o1 = np.asarray(results[i]["out1"], np.float32)     # (128, 4*KP)
        o2 = np.asarray(results[i]["out2"], np.float32)     # (128, 8*KP)
        Fo[i * NS:(i + 1) * NS] = (
            o1.reshape(128, 4, KP).transpose(1, 0, 2).reshape(NS, KP))
        Fo[N + i * MS:N + (i + 1) * MS] = (
            o2.reshape(128, 8, KP).transpose(1, 0, 2).reshape(MS, KP))
        o3 = o3 + np.asarray(results[i]["out3"], np.float32)[0]
    Fo[N + M] = o3
    return Fo


def _run_sharded(inputs, trace=False, trace_kwargs=None):
    from concourse import bass_utils
    in_maps, kta, c_inv, xPx, wt = _prep(**inputs)
    nc = _get_nc((kta, G2_DR, c_inv))
    res = bass_utils.run_bass_kernel_spmd(
        nc, in_maps, core_ids=list(range(NC)), trace=trace,
        **(trace_kwargs or {}))
    return _assemble(res.results, xPx, wt), res


def kernel(**inputs) -> np.ndarray:
    out, _ = _run_sharded(inputs, trace=False)
    return out
